# revision 4
# baseline (speedup 1.0000x reference)
import functools

import jax
import jax.numpy as jnp
import numpy as np

# Baseline factored routing + three deltas, keeping baseline einsum
# layouts (which the Neuron backend lowers well):
#  1. ones-channel fold: i 8->9 with xg1[...,8,:]=1 and Wt[...,8]=bias,
#     so h / cs / bias einsums and their broadcast-adds disappear.
#  2. no max-shift in softmax (|L| = O(1), exp cannot overflow).
#  3. bf16 operands on the two big (b,s)-batched einsums (dL, y) and the
#     small v/W einsums, fp32 accumulation where it matters.

NUM_SHARED = 32
IN_DIM = 8
NUM_OUT = 10
OUT_DIM = 16
ROUTE_NUM = 3
EPS = 1e-20

N_CORES = 8
BS = 256
H = 6
P = H * H
I = NUM_SHARED * P

F32 = jnp.float32
BF16 = jnp.bfloat16


def _squash(s):
    n2 = jnp.sum(s * s, axis=2, keepdims=True)
    n = jnp.sqrt(n2)
    return s * (n2 / (1.0 + n2) / (n + EPS))


def _caps_shard(x, w, b_conv):
    bs = x.shape[0]
    S, J, D, Di = NUM_SHARED, NUM_OUT, OUT_DIM, IN_DIM
    xg = x.reshape(bs, S, Di, P)
    ones = jnp.ones((bs, S, 1, P), x.dtype)
    xg1 = jnp.concatenate([xg, ones], axis=2).astype(BF16)   # (b, s, 9, p)

    Wr = w.reshape(S, J, D, Di)
    Br = b_conv.reshape(S, J, D)
    Wt = jnp.concatenate([Wr, Br[..., None]], axis=3).astype(BF16)  # (s, j, d, 9)

    # r0: uniform c
    xs0 = jnp.sum(xg1, axis=3, dtype=F32).astype(BF16)       # (b, s, 9)
    s0 = jnp.einsum('bsi,sjdi->bjd', xs0, Wt, preferred_element_type=F32)
    v = _squash(s0 * (1.0 / I))

    L = None
    for r in range(1, ROUTE_NUM):
        g1 = jnp.einsum('bjd,sjdi->bjsi', v.astype(BF16), Wt,
                        preferred_element_type=BF16)          # (b, j, s, 9)
        dL = jnp.einsum('bjsi,bsip->bjsp', g1, xg1,
                        preferred_element_type=F32)           # (b, j, s, p)
        L = dL if L is None else L + dL

        Lf = L.reshape(bs, J, I)
        e = jnp.exp(Lf)
        c = (e / jnp.sum(e, axis=2, keepdims=True)).reshape(bs, J, S, P)
        y1 = jnp.einsum('bjsp,bsip->bjsi', c.astype(BF16), xg1,
                        preferred_element_type=BF16)          # (b, j, s, 9)
        s_r = jnp.einsum('bjsi,sjdi->bjd', y1, Wt,
                         preferred_element_type=F32)
        v = _squash(s_r)
    return v.astype(F32)


@functools.cache
def _pmapped(n_cores: int):
    return jax.pmap(_caps_shard, axis_name='cores', devices=jax.devices()[:n_cores])


def kernel(x: np.ndarray, w: np.ndarray, b_conv: np.ndarray) -> np.ndarray:
    bs = x.shape[0]
    n_cores = N_CORES
    n_dev = len(jax.devices())
    while n_cores > 1 and (n_cores > n_dev or bs % n_cores != 0):
        n_cores //= 2
    shard = bs // n_cores
    xs = np.ascontiguousarray(x.reshape(n_cores, shard, *x.shape[1:]))
    ws = np.ascontiguousarray(np.broadcast_to(w, (n_cores,) + w.shape))
    bs_ = np.ascontiguousarray(np.broadcast_to(b_conv, (n_cores,) + b_conv.shape))
    v = _pmapped(n_cores)(xs, ws, bs_)
    v = np.asarray(v)
    return v.reshape(bs, NUM_OUT, OUT_DIM)
